# revision 67
# baseline (speedup 1.0000x reference)
"""LConv (7x7 position-linear conv) Trainium2 Bass kernel.

Full inputs in, full output out. Sharding: data-parallel over batch,
16 images -> 8 NeuronCores (2 images/core). abc/bias replicated.

Math (K=7, S=1, P=3, D=1, G=1, C=COUT=128):
  out[o,i,j] = sum_c sum_{t=1..7} P_t[c,o]*W1[c,i+t,j] + B[c,o]*W2[c,i+t,j]
             + bias[o]
  W1 = 7-wide box sum along W of padded x; W2 = position-ramp sum along W.
  Both are computed from running sums (cumsum) along each padded row:
  rows are stored 122 wide = 7 lead zeros + 112 data + 3 trail zeros, so
  cumsum(stream shifted by 7) - cumsum(stream) telescopes exactly to the
  7-tap box (the 7 lead zeros kill the stream-prefix offset).
  P_t = (t-4)*A + C ; A=abc[0:128], B=abc[128:256], C=abc[256:384].

The H-direction 7-tap box of W2 uses quad sums:
  box7(i) = q[i+1] + q[i+5] - w2[i+8],  q[r] = w2[r]+w2[r+1]+w2[r+2]+w2[r+3]
so the B-side needs 3 matmuls per 4-row half instead of 7 (10 total/half).

Pipeline layout (v3):
  - x slabs DMA'd CONTIGUOUSLY ([128,16*112], full-BW descriptors), then
    scattered into the 122-grid by the Activation engine (copy).
  - jmap multiply runs on Pool into a separate xj buffer so the two DVE
    scans run back-to-back.
  - stage-2 matmuls interleaved into the slab loop (8-row output groups,
    two 4-row PSUM-bank halves); drains (bias + store) lag one step so
    their PE waits never head-block the Act queue. PE runs gap-free and
    pipelines across the two images per core.
  - image 0's first four slabs are produced in 8-row halves (slab 0's
    DMA even in 4-row quarters) so groups come ready a half-slab apart
    through the production-bound fill; slab 1's DMAs prefetch in the
    preamble. The PE-throughput chain and the production chain are
    co-critical at ~118.8 us in the CoreSim cost model.
"""

import numpy as np

import concourse.bacc as bacc
import concourse.mybir as mybir
from concourse import tile
from concourse.bass_utils import run_bass_kernel_spmd

F32 = mybir.dt.float32
BF16 = mybir.dt.bfloat16
I32 = mybir.dt.int32
AF = mybir.ActivationFunctionType
ALU = mybir.AluOpType

B_TOT = 16
N_CORES = 8
B_PER = B_TOT // N_CORES
CIN = 128
COUT = 128
H = W = 112
PW2 = 122                 # row layout: 7 lead zeros + 112 data + 3 trail
LEAD = 7
RLEAD = 4                 # lead pad rows in the streams (stream row -4..-1)
PH1 = 119                 # w1/q rows: stream -4..114
PH2 = 120                 # w2 rows: stream -4..115 (t=8 tap reads row 115)
ROWS_PER_SLAB = 16
N_SLABS = H // ROWS_PER_SLAB
SLAB_FREE = ROWS_PER_SLAB * PW2        # 1952
SLAB_DATA = ROWS_PER_SLAB * W          # 1792
DSTREAM = SLAB_FREE - 7                # 1945
W1FULL = PH1 * PW2                     # 14518
W2FULL = PH2 * PW2                     # 14640
GROUP_ROWS = 8
N_GROUPS = H // GROUP_ROWS             # 14
GROUP_FREE = GROUP_ROWS * W            # 896
HALF_ROWS = 4
HALF_FREE = HALF_ROWS * W              # 448

# group i0 is ready once slab s is produced and its q rows exist:
# w1 stream rows <= i0+10, q rows <= i0+8, w2 rows <= i0+11 -> i0 <= 16s+4
_SCHED = {s: [] for s in range(N_SLABS)}
_done = 0
for _s in range(N_SLABS):
    while _done < N_GROUPS and _done * GROUP_ROWS <= 16 * _s + 4:
        _SCHED[_s].append(_done * GROUP_ROWS)
        _done += 1
_POST = [g * GROUP_ROWS for g in range(_done, N_GROUPS)]

_CACHE = {}


def _register_opa():
    from concourse.dve_spec import Spec, Src0, Src1, scan, AluOp, lower
    import concourse.dve_ops as dve_ops
    from concourse.dve_uop import DveOpSpec

    if any(op.name == "BOXDIFF7" for op in dve_ops.OPS):
        return next(op for op in dve_ops.OPS if op.name == "BOXDIFF7")
    spec = Spec(
        body=scan(AluOp.ADD, Src0) - scan(AluOp.ADD, Src1),
        reference=lambda in0, in1, *c: (
            np.cumsum(in0, axis=-1) - np.cumsum(in1, axis=-1)
        ),
    )
    row = dve_ops._CUSTOM_DVE_ROW_BASE + len(dve_ops.OPS)
    shas = {}
    for ver in ("v3", "v4"):
        s = DveOpSpec(
            name="BOXDIFF7", opcode=row, uops=lower(spec, ver=ver), rd1_en=True
        )
        shas[ver] = s.sha(ver)
    op = dve_ops.DveOp("BOXDIFF7", spec, subdim=False, uops_sha=shas)
    dve_ops.OPS.append(op)
    dve_ops._SUB_OPCODE_FOR_NAME[op.name] = row
    dve_ops.CUSTOM_DVE_SPECS[op.name] = op.spec
    return op


def _build():
    nc = bacc.Bacc("TRN2", target_bir_lowering=False, debug=False)
    opa = _register_opa()

    t_x = nc.dram_tensor("xs", [B_PER, CIN, H, W], F32, kind="ExternalInput")
    t_pw = nc.dram_tensor("pw", [7, CIN, COUT], F32, kind="ExternalInput")
    t_bw = nc.dram_tensor("bw", [CIN, COUT], F32, kind="ExternalInput")
    t_bias = nc.dram_tensor("bias", [COUT, 1], F32, kind="ExternalInput")
    t_out = nc.dram_tensor("out", [B_PER, COUT, H, W], F32, kind="ExternalOutput")

    with tile.TileContext(nc) as tc:
        with (
            tc.tile_pool(name="const", bufs=1) as cpool,
            tc.tile_pool(name="wfull", bufs=1) as wpool,
            tc.tile_pool(name="slab", bufs=2) as spool,
            tc.tile_pool(name="scr", bufs=1) as rpool,
            tc.tile_pool(name="outs", bufs=3) as opool,
            tc.tile_pool(name="ps", bufs=4, space="PSUM") as ppool,
        ):
            # ---- tiles ----
            pw_f = cpool.tile([CIN, 7 * COUT], F32, tag="pwf")
            pw = cpool.tile([CIN, 7 * COUT], BF16, tag="pwb")
            bw_f = cpool.tile([CIN, COUT], F32, tag="bwf")
            bw = cpool.tile([CIN, COUT], BF16, tag="bwb")
            bwn = cpool.tile([CIN, COUT], BF16, tag="bwn")
            bias_sb = cpool.tile([COUT, 1], F32, tag="bias")
            jmap = cpool.tile([128, SLAB_FREE], F32, tag="jmap")
            jp4 = cpool.tile([128, SLAB_DATA], F32, tag="jp4")
            w1 = wpool.tile([CIN, W1FULL], BF16, tag="w1")
            w2 = wpool.tile([CIN, W2FULL], BF16, tag="w2")
            qs = wpool.tile([CIN, W1FULL], BF16, tag="qs")
            xp_bufs = []
            for i in range(2):
                xpb = spool.tile([CIN, SLAB_FREE], F32, tag=f"xp{i}")
                xp_bufs.append(xpb)

            # ---- preambles, ordered for minimal pipeline lead-in ----
            # tiny dummy activation so the one-time LoadActFuncSet runs at
            # t~0 instead of delaying the first scatter
            warm1 = cpool.tile([1, 1], F32, tag="warm1")
            nc.vector.memset(warm1[:], 0.0)
            nc.scalar.activation(warm1[:], warm1[:], AF.Identity, scale=1.0)
            # SP: first input slab in 4-row quarters (so the first scatter
            # only waits a quarter slab), then the small constants
            xc0 = spool.tile([CIN, SLAB_DATA], F32, tag="xc")
            half = ROWS_PER_SLAB // 2
            quarter = ROWS_PER_SLAB // 4
            for kq in range(4):
                nc.sync.dma_start(
                    xc0[:, kq * quarter * W : (kq + 1) * quarter * W].rearrange(
                        "c (r j) -> c r j", j=W
                    ),
                    t_x[0, :, kq * quarter : (kq + 1) * quarter, :],
                )
            nc.sync.dma_start(
                pw_f[:].rearrange("c (t o) -> c t o", t=7),
                t_pw[:].transpose([1, 0, 2]),
            )
            xc1 = spool.tile([CIN, SLAB_DATA], F32, tag="xc")
            for kh in range(2):
                nc.sync.dma_start(
                    xc1[:, kh * half * W : (kh + 1) * half * W].rearrange(
                        "c (r j) -> c r j", j=W
                    ),
                    t_x[0, :, 16 + kh * half : 16 + (kh + 1) * half, :],
                )
            nc.sync.dma_start(bw_f[:], t_bw[:])
            nc.sync.dma_start(bias_sb[:], t_bias[:])
            # Pool: position ramps, directly in fp32 (values <= 118, exact;
            # col map value (p-3) at col p of the 122-grid == data col + 4,
            # matching the recenter term (j+4))
            nc.gpsimd.iota(
                jmap[:], pattern=[[0, ROWS_PER_SLAB], [1, PW2]],
                base=-3, channel_multiplier=0,
                allow_small_or_imprecise_dtypes=True,
            )
            nc.gpsimd.iota(
                jp4[:], pattern=[[0, ROWS_PER_SLAB], [1, W]],
                base=4, channel_multiplier=0,
                allow_small_or_imprecise_dtypes=True,
            )
            # DVE: xp zero-fill (gap cols must be 0 before first scatter)
            for xpb in xp_bufs:
                nc.vector.memset(xpb[:], 0.0)
            # Pool: stream pad-row zero fills (lead rows feed group 0)
            nc.gpsimd.memset(w1[:, : RLEAD * PW2], 0.0)
            nc.gpsimd.memset(w2[:, : RLEAD * PW2], 0.0)
            nc.gpsimd.memset(w1[:, (RLEAD + H) * PW2 :], 0.0)
            nc.gpsimd.memset(w2[:, (RLEAD + H) * PW2 :], 0.0)
            def row_view(buf, r0, nrows=ROWS_PER_SLAB):
                # strided (nrows,112) view at data cols of the 122-grid
                base = (RLEAD + r0) * PW2
                return buf[:, base : base + nrows * PW2].rearrange(
                    "c (r q) -> c r q", q=PW2
                )[:, :, 3:115]

            def rhs(buf, i0h, trow):
                base = (i0h + trow) * PW2
                return buf[:, base : base + HALF_ROWS * PW2].rearrange(
                    "c (r q) -> c r q", q=PW2
                )[:, :, 3:115]

            def emit_mm(b, i0):
                # two 4-row halves, each in its own PSUM bank (a matmul
                # target must not cross the 2 KiB bank line)
                acc = ppool.tile([COUT, 1024], F32, tag="acc")
                for h in range(2):
                    i0h = i0 + HALF_ROWS * h
                    accv = acc[:, h * 512 : h * 512 + HALF_FREE]
                    for t in range(1, 8):
                        nc.tensor.matmul(
                            accv,
                            pw[:, (t - 1) * COUT : t * COUT],
                            rhs(w1, i0h, t),
                            start=(t == 1),
                            stop=False,
                        )
                    for t in (1, 5):
                        nc.tensor.matmul(
                            accv, bw[:], rhs(qs, i0h, t), start=False, stop=False
                        )
                    nc.tensor.matmul(
                        accv, bwn[:], rhs(w2, i0h, 8), start=False, stop=True
                    )
                return acc

            def drain_group(acc, b, i0, split=False):
                if split:
                    # per-half drain shortens the kernel tail
                    for h in range(2):
                        i0h = i0 + HALF_ROWS * h
                        ot = opool.tile([COUT, GROUP_FREE], F32, tag="ot")
                        ot = ot[:, :HALF_FREE]
                        nc.scalar.activation(
                            ot, acc[:, h * 512 : h * 512 + HALF_FREE],
                            AF.Identity, bias=bias_sb[:], scale=1.0,
                        )
                        nc.sync.dma_start(
                            t_out[b, :, i0h : i0h + HALF_ROWS, :].rearrange(
                                "o r j -> o (r j)"
                            ),
                            ot[:],
                        )
                    return
                ot = opool.tile([COUT, GROUP_FREE], F32, tag="ot")
                nc.scalar.activation(
                    ot[:].rearrange("c (g q) -> c g q", g=2),
                    acc[:].rearrange("c (g q) -> c g q", g=2)[:, :, :HALF_FREE],
                    AF.Identity,
                    bias=bias_sb[:],
                    scale=1.0,
                )
                nc.sync.dma_start(
                    t_out[b, :, i0 : i0 + GROUP_ROWS, :].rearrange(
                        "o r j -> o (r j)"
                    ),
                    ot[:],
                )

            def quad_pass(qlo, qhi):
                # q[r] = w2[r]+w2[r+1]+w2[r+2]+w2[r+3] for r in [qlo, qhi]
                n = qhi - qlo + 1
                ps = rpool.tile([CIN, (ROWS_PER_SLAB + 2) * W], BF16, tag="pq")
                pv = ps[:, : (n + 2) * W].rearrange("c (r j) -> c r j", j=W)
                nc.gpsimd.tensor_tensor(
                    pv, row_view(w2, qlo, n + 2), row_view(w2, qlo + 1, n + 2),
                    op=ALU.add,
                )
                nc.gpsimd.tensor_tensor(
                    row_view(qs, qlo, n), pv[:, :n], pv[:, 2 : n + 2],
                    op=ALU.add,
                )

            def produce(b, xp, xrow0, r0, nrows, xc, xcoff, dma=True, first=False):
                """Row-filter stage for `nrows` image rows starting at r0,
                landing in xp grid rows xrow0.. and streams rows r0.."""
                nfree = nrows * PW2
                ndstr = nfree - 7
                ndata = nrows * W
                xcv = xc[:, xcoff * W : (xcoff + nrows) * W]
                if dma:
                    nc.sync.dma_start(
                        xcv.rearrange("c (r j) -> c r j", j=W),
                        t_x[b, :, r0 : r0 + nrows, :],
                    )
                xpv = xp[:, xrow0 * PW2 : xrow0 * PW2 + nfree]
                # scatter 112-wide rows into the 122-grid (Act engine)
                nc.scalar.copy(
                    xpv.rearrange("c (r q) -> c r q", q=PW2)[
                        :, :, LEAD : LEAD + W
                    ],
                    xcv.rearrange("c (r j) -> c r j", j=W),
                )
                if first:
                    # weight casts on Act, behind the first scatter but
                    # ahead of the first w1 copy, so the first matmul is
                    # gated by the stream copy alone
                    nc.scalar.copy(pw[:], pw_f[:])
                    nc.scalar.copy(bw[:], bw_f[:])
                    nc.scalar.activation(bwn[:], bw_f[:], AF.Identity, scale=-1.0)
                # position-weighted copy on Pool (in parallel with scan1)
                xj = rpool.tile([CIN, SLAB_FREE], F32, tag="xj")
                xjv = xj[:, xrow0 * PW2 : xrow0 * PW2 + nfree]
                nc.gpsimd.tensor_tensor(xjv, xpv, jmap[:, :nfree], op=ALU.mult)

                w1s = w1[:, (RLEAD + r0) * PW2 : (RLEAD + r0) * PW2 + ndstr]
                d1 = rpool.tile([CIN, SLAB_FREE], F32, tag="d1")
                d1v = d1[:, xrow0 * PW2 : xrow0 * PW2 + ndstr]
                nc.vector._custom_dve(
                    opa, out=d1v, in0=xpv[:, 7:], in1=xpv[:, :ndstr]
                )
                nc.scalar.copy(w1s, d1v)
                rawd = rpool.tile([CIN, SLAB_FREE], F32, tag="rawd")
                rawdv = rawd[:, xrow0 * PW2 : xrow0 * PW2 + ndstr]
                nc.vector._custom_dve(
                    opa, out=rawdv, in0=xjv[:, 7:], in1=xjv[:, :ndstr]
                )
                # w2b = (j+4) * W1  (gpsimd, fp32, strided d1 view)
                d1g = d1[:, xrow0 * PW2 : xrow0 * PW2 + nfree].rearrange(
                    "c (r q) -> c r q", q=PW2
                )[:, :, 3:115]
                w2b = rpool.tile([CIN, SLAB_DATA], F32, tag="w2b")
                w2bv = w2b[:, : nrows * W].rearrange("c (r j) -> c r j", j=W)
                nc.gpsimd.tensor_tensor(
                    w2bv,
                    jp4[:, :ndata].rearrange("c (r j) -> c r j", j=W),
                    d1g,
                    op=ALU.mult,
                )
                # w2 = rawd - w2b (bf16 cast on write; values are small)
                rawg = rawd[:, xrow0 * PW2 : xrow0 * PW2 + nfree].rearrange(
                    "c (r q) -> c r q", q=PW2
                )[:, :, 3:115]
                nc.vector.tensor_tensor(
                    row_view(w2, r0, nrows), rawg, w2bv, op=ALU.subtract
                )

            # Matmuls go out the same step their data is ready; drains
            # (bias + store, on Act/SP) lag one step so their PE waits
            # never head-block the next slab's scatter/copy in the
            # in-order Act queue.
            state = {"dr": []}

            def step(ready):
                prev = state["dr"]
                for acc, bb, i0 in prev:
                    drain_group(acc, bb, i0)
                state["dr"] = [(emit_mm(bb, i0), bb, i0) for bb, i0 in ready]

            for b in range(B_PER):
                for s in range(N_SLABS):
                    r0 = s * ROWS_PER_SLAB
                    xp = xp_bufs[s % 2]
                    if b == 0 and s == 0:
                        # split the first slab into two 8-row halves to
                        # shorten the production chain ahead of group 0
                        # (DMAs already issued in the preamble)
                        produce(b, xp, 0, 0, half, xc0, 0, dma=False,
                                first=True)
                        produce(b, xp, half, half, half, xc0, half, dma=False)
                        quad_pass(-3, half - 4)
                        quad_pass(half - 3, 12)
                    elif b == 0 and s in (1, 2, 3):
                        # early slabs in halves as well: their two groups
                        # come ready a half-slab apart, keeping PE fed
                        # through the production-bound fill phase
                        if s == 1:
                            xc = xc1
                        else:
                            xc = spool.tile([CIN, SLAB_DATA], F32, tag="xc")
                        produce(b, xp, 0, r0, half, xc, 0, dma=(s != 1))
                        quad_pass(16 * s - 3, 16 * s + 4)
                        step([(b, 16 * s - 8)])
                        produce(b, xp, half, r0 + half, half, xc, half,
                                dma=(s != 1))
                        quad_pass(16 * s + 5, 16 * s + 12)
                        step([(b, 16 * s)])
                        continue
                    else:
                        xc = spool.tile([CIN, SLAB_DATA], F32, tag="xc")
                        produce(b, xp, 0, r0, ROWS_PER_SLAB, xc, 0)
                        # quad rows available from this slab (<= r0+15)
                        quad_pass(16 * s - 3, 16 * s + 12)
                    step([(b, i0) for i0 in _SCHED[s]])
                # image epilogue: trailing quads over the pad rows
                quad_pass(16 * N_SLABS - 3, H)
                step([(b, i0) for i0 in _POST])
            # flush the pipeline
            for acc, bb, i0 in state["dr"]:
                drain_group(acc, bb, i0, split=True)

    nc.compile()
    return nc


def kernel(x: np.ndarray, abc: np.ndarray, bias: np.ndarray) -> np.ndarray:
    x = np.ascontiguousarray(x, dtype=np.float32)
    abc = np.asarray(abc, dtype=np.float32)
    bias = np.asarray(bias, dtype=np.float32)

    if "nc" not in _CACHE:
        _CACHE["nc"] = _build()
    nc = _CACHE["nc"]

    A, Bm, Cc = abc[0:128], abc[128:256], abc[256:384]
    pw = np.stack([(t - 4.0) * A + Cc for t in range(1, 8)]).astype(np.float32)
    in_maps = []
    for c in range(N_CORES):
        in_maps.append(
            {
                "xs": x[c * B_PER : (c + 1) * B_PER],
                "pw": pw,
                "bw": np.ascontiguousarray(Bm),
                "bias": np.ascontiguousarray(bias.reshape(COUT, 1)),
            }
        )
    res = run_bass_kernel_spmd(nc, in_maps, list(range(N_CORES)))
    out = np.concatenate([res.results[c]["out"] for c in range(N_CORES)], axis=0)
    return out.astype(np.float32)


if __name__ == "__main__":
    rng = np.random.default_rng(0)
    x = rng.standard_normal((16, 128, 112, 112), dtype=np.float32)
    abc = (rng.standard_normal((384, 128)) * 0.05).astype(np.float32)
    bias = (rng.standard_normal((128,)) * 0.05).astype(np.float32)
    out = kernel(x=x, abc=abc, bias=bias)
    print(out.shape, out.dtype)
